# revision 8
# baseline (speedup 1.0000x reference)
"""Multi-head MLP (additive) attention on 8 TRN2 NeuronCores.

Data-parallel over batch: B=32 -> 4 batches per core. Each core computes
its shard fully (no collectives); host scatters inputs / gathers outputs.

Per-core pipeline (b = local batch, L=2048 keys, Dm=1024, H=16, d=64):
  key_up^T[c]  = Wk[:,c].T @ key^T          (f32r matmuls, PSUM accum)
  tanh_t       = tanh(key_up^T + (q@Wq+bq+bk)^T)   (ScalarE, fused bias)
  scores[2c]   = W2.T @ tanh_t              (per-head dot with w_score)
  attn         = softmax(scores + mask*-1e30)      (DVE/ACT, 16 partitions)
  ctx_full     = attn @ value               (wasteful [16,1024], diag-masked)
  out          = ctx_flat @ Wf + bf         (batched over 4 local b)

b_score is dropped: softmax is shift-invariant.
"""

import numpy as np

import concourse.bass as bass
import concourse.mybir as mybir
import concourse.tile as tile
from concourse import bacc
from concourse.bass_utils import run_bass_kernel_spmd
from concourse.masks import make_identity

F32 = mybir.dt.float32
F32R = mybir.dt.float32r
AF = mybir.ActivationFunctionType
ALU = mybir.AluOpType
AX = mybir.AxisListType

NCORES = 8
B, LK, DK, DM, H = 32, 2048, 1024, 1024, 16
BL = B // NCORES  # 4 local batches per core


def build_program():
    nc = bacc.Bacc("TRN2", target_bir_lowering=False, num_devices=NCORES)

    keyT_d = nc.dram_tensor("keyT", [BL, DK, LK], F32R, kind="ExternalInput").ap()
    val_d = nc.dram_tensor("value", [BL, LK, DK], F32R, kind="ExternalInput").ap()
    q_d = nc.dram_tensor("query", [BL, DM], F32, kind="ExternalInput").ap()
    mneg_d = nc.dram_tensor("maskneg", [BL, H, LK], F32, kind="ExternalInput").ap()
    wk_d = nc.dram_tensor("Wk", [DK, DM], F32R, kind="ExternalInput").ap()
    wq_d = nc.dram_tensor("Wq", [DM, DM], F32R, kind="ExternalInput").ap()
    wf_d = nc.dram_tensor("Wf", [DK, DK], F32R, kind="ExternalInput").ap()
    w2_d = nc.dram_tensor("w16", [128, 8 * H], F32R, kind="ExternalInput").ap()
    qkb_d = nc.dram_tensor("qkb4", [BL, DM], F32, kind="ExternalInput").ap()
    bf4_d = nc.dram_tensor("bf4", [BL, DK], F32, kind="ExternalInput").ap()
    diag_d = nc.dram_tensor("diagmask", [H, DK], F32, kind="ExternalInput").ap()

    out_d = nc.dram_tensor("out", [BL, DK], F32, kind="ExternalOutput").ap()
    attn_d = nc.dram_tensor("attn", [BL, H, LK], F32, kind="ExternalOutput").ap()

    with tile.TileContext(nc) as tc:
        _body(tc, nc, keyT_d, val_d, q_d, mneg_d, wk_d, wq_d, wf_d, w2_d,
              qkb_d, bf4_d, diag_d, out_d, attn_d)

    nc.compile()
    return nc


def _body(tc, nc, keyT_d, val_d, q_d, mneg_d, wk_d, wq_d, wf_d, w2_d,
          qkb_d, bf4_d, diag_d, out_d, attn_d):
    from contextlib import ExitStack

    ctx = ExitStack()
    const = ctx.enter_context(tc.tile_pool(name="const", bufs=1))
    bigw2 = ctx.enter_context(tc.tile_pool(name="bigw2", bufs=1))
    wkp = ctx.enter_context(tc.tile_pool(name="wkp", bufs=1))
    ktp = ctx.enter_context(tc.tile_pool(name="ktp", bufs=2))
    vp = ctx.enter_context(tc.tile_pool(name="vp", bufs=5))
    thp = ctx.enter_context(tc.tile_pool(name="thp", bufs=3))
    scp = ctx.enter_context(tc.tile_pool(name="scp", bufs=2))
    atp = ctx.enter_context(tc.tile_pool(name="atp", bufs=2))
    aTp = ctx.enter_context(tc.tile_pool(name="aTp", bufs=3))
    mkp = ctx.enter_context(tc.tile_pool(name="mkp", bufs=2))
    smp = ctx.enter_context(tc.tile_pool(name="smp", bufs=4))
    cmp_ = ctx.enter_context(tc.tile_pool(name="cmp", bufs=2))
    pku = ctx.enter_context(tc.tile_pool(name="pku", bufs=2, space="PSUM"))
    psc = ctx.enter_context(tc.tile_pool(name="psc", bufs=2, space="PSUM"))
    pcx = ctx.enter_context(tc.tile_pool(name="pcx", bufs=2, space="PSUM"))

    # ---- constants ----
    ident = const.tile([128, 128], F32)
    make_identity(nc, ident[:])
    w16_sb = const.tile([128, 8 * H], F32R)
    nc.sync.dma_start(w16_sb[:], w2_d[:])
    diag_sb = const.tile([H, DK], F32)
    nc.sync.dma_start(diag_sb[:], diag_d[:])
    qkb_sb = const.tile([BL, DM], F32)
    nc.sync.dma_start(qkb_sb[:], qkb_d[:])
    bf4_sb = const.tile([BL, DK], F32)
    nc.sync.dma_start(bf4_sb[:], bf4_d[:])
    qTb = const.tile([128, 8 * BL], F32)   # tanh bias, [128, c*4+b]
    ctxT = const.tile([128, 8 * BL], F32)  # ctx_flat^T, [128, c*4+b]

    # ---- query projection (prologue) ----
    wq_sb = bigw2.tile([128, 8 * DM], F32R, tag="bigw2", name="wq_sb")
    for k in range(8):
        nc.sync.dma_start(wq_sb[:, k * DM:(k + 1) * DM], wq_d[k * 128:(k + 1) * 128, :])
    q_sb = const.tile([BL, DM], F32)
    nc.sync.dma_start(q_sb[:], q_d[:])

    qT = const.tile([128, 8 * BL], F32R)
    for c in range(8):
        pt = pku.tile([128, 512], F32, tag="t", name="pt_q")
        nc.tensor.transpose(pt[:, :BL], q_sb[:, c * 128:(c + 1) * 128], ident[:BL, :BL])
        nc.any.tensor_copy(out=qT[:, c * BL:(c + 1) * BL], in_=pt[:, :BL])

    psq = pcx.tile([BL, DM], F32, tag="c", name="psq")
    for c in range(8):
        for hf in range(2):
            nc.tensor.matmul(
                psq[:, hf * 512:(hf + 1) * 512],
                qT[:, c * BL:(c + 1) * BL],
                wq_sb[:, c * DM + hf * 512: c * DM + (hf + 1) * 512],
                start=(c == 0), stop=(c == 7),
            )
    qsum = const.tile([BL, DM], F32)
    nc.vector.tensor_tensor(qsum[:], psq[:], qkb_sb[:], ALU.add)
    for c in range(8):
        pt = pku.tile([128, 512], F32, tag="t", name="pt_qb")
        nc.tensor.transpose(pt[:, :BL], qsum[:, c * 128:(c + 1) * 128], ident[:BL, :BL])
        nc.any.tensor_copy(out=qTb[:, c * BL:(c + 1) * BL], in_=pt[:, :BL])

    # ---- Wk ----
    wk_sb = wkp.tile([128, 8 * DM], F32R)
    for k in range(8):
        nc.sync.dma_start(wk_sb[:, k * DM:(k + 1) * DM], wk_d[k * 128:(k + 1) * 128, :])

    # ---- main loop over local batches ----
    for b in range(BL):
        mneg = mkp.tile([H, LK], F32)
        nc.sync.dma_start(mneg[:], mneg_d[b])
        scores = scp.tile([H, LK], F32)

        for j in range(4):  # L chunks of 512
            kT = ktp.tile([128, 8 * 512], F32R, tag="kT")
            for k in range(8):
                nc.sync.dma_start(
                    kT[:, k * 512:(k + 1) * 512],
                    keyT_d[b, k * 128:(k + 1) * 128, j * 512:(j + 1) * 512],
                )
            ps = psc.tile([H, 512], F32, tag="s")
            for c in range(8):
                pk = pku.tile([128, 512], F32, tag="t", name="pk")
                for k in range(8):
                    nc.tensor.matmul(
                        pk[:],
                        wk_sb[:, k * DM + c * 128: k * DM + (c + 1) * 128],
                        kT[:, k * 512:(k + 1) * 512],
                        start=(k == 0), stop=(k == 7),
                    )
                th = thp.tile([128, 512], F32R)
                nc.scalar.activation(
                    th[:], pk[:], AF.Tanh,
                    bias=qTb[:, c * BL + b: c * BL + b + 1], scale=1.0,
                )
                nc.tensor.matmul(
                    ps[:], w16_sb[:, c * H:(c + 1) * H], th[:],
                    start=(c == 0), stop=(c == 7),
                )
            nc.vector.tensor_tensor(
                scores[:, j * 512:(j + 1) * 512], ps[:],
                mneg[:, j * 512:(j + 1) * 512], ALU.add,
            )

        # softmax over L (free axis), 16 head-partitions
        negmax = smp.tile([H, 1], F32)
        nc.vector.reduce_max(negmax[:], scores[:], axis=AX.X, negate=True)
        attn = atp.tile([H, LK], F32)
        sums = smp.tile([H, 1], F32)
        nc.scalar.activation(attn[:], scores[:], AF.Exp, bias=negmax[:],
                             accum_out=sums[:])
        recip = smp.tile([H, 1], F32)
        nc.vector.reciprocal(recip[:], sums[:])
        nc.vector.tensor_scalar_mul(attn[:], attn[:], recip[:])
        nc.sync.dma_start(attn_d[b], attn[:])

        # context: ctx_full[16,1024] = sum_j2 attnT[j2].T @ value[j2]
        pc = pcx.tile([H, DK], F32, tag="c", name="pc")
        for j2 in range(16):
            pt = pku.tile([128, 512], F32, tag="t", name="pt_a")
            nc.tensor.transpose(
                pt[:, :H], attn[:, j2 * 128:(j2 + 1) * 128], ident[:H, :H])
            aT = aTp.tile([128, H], F32R)
            nc.any.tensor_copy(out=aT[:], in_=pt[:, :H])
            v = vp.tile([128, DK], F32R)
            nc.sync.dma_start(v[:], val_d[b, j2 * 128:(j2 + 1) * 128, :])
            for hf in range(2):
                nc.tensor.matmul(
                    pc[:, hf * 512:(hf + 1) * 512], aT[:],
                    v[:, hf * 512:(hf + 1) * 512],
                    start=(j2 == 0), stop=(j2 == 15),
                )
        ctxm = cmp_.tile([H, DK], F32)
        nc.vector.tensor_tensor(ctxm[:], pc[:], diag_sb[:], ALU.mult)
        for c in range(8):
            pt = pku.tile([128, 512], F32, tag="t", name="pt_c")
            nc.tensor.transpose(
                pt[:, :H], ctxm[:, c * 128:(c + 1) * 128], ident[:H, :H])
            nc.vector.reduce_sum(
                ctxT[:, c * BL + b: c * BL + b + 1], pt[:, :H], axis=AX.X)

    # rounded copy of ctxT for the final f32r matmuls
    ctxTr = const.tile([128, 8 * BL], F32R)
    nc.any.tensor_copy(out=ctxTr[:], in_=ctxT[:])

    # ---- final projection: out[4,1024] = ctx_flat @ Wf + bf ----
    wf_sb = bigw2.tile([128, 8 * DK], F32R, tag="bigw2", name="wf_sb")
    for k in range(8):
        nc.sync.dma_start(wf_sb[:, k * DK:(k + 1) * DK], wf_d[k * 128:(k + 1) * 128, :])
    po = pcx.tile([BL, DK], F32, tag="c", name="po")
    for c in range(8):
        for hf in range(2):
            nc.tensor.matmul(
                po[:, hf * 512:(hf + 1) * 512],
                ctxTr[:, c * BL:(c + 1) * BL],
                wf_sb[:, c * DK + hf * 512: c * DK + (hf + 1) * 512],
                start=(c == 0), stop=(c == 7),
            )
    out_sb = const.tile([BL, DK], F32)
    nc.vector.tensor_tensor(out_sb[:], po[:], bf4_sb[:], ALU.add)
    nc.sync.dma_start(out_d[:], out_sb[:])

    ctx.close()


_CACHED_NC = None


def _get_nc():
    global _CACHED_NC
    if _CACHED_NC is None:
        _CACHED_NC = build_program()
    return _CACHED_NC


def round_f32r(x):
    u = np.ascontiguousarray(x, np.float32).view(np.uint32)
    u = (u + np.uint32(0x800)) & np.uint32(0xFFFFF000)
    return u.view(np.float32)


def prep_in_maps(key, value, query, mask, Wk, bk, Wq, bq, w_score, b_score,
                 Wf, bf):
    key = np.ascontiguousarray(key, dtype=np.float32)
    value = round_f32r(value)
    query = np.ascontiguousarray(query, dtype=np.float32)

    w16 = np.zeros((8, 128, H), np.float32)
    for c in range(8):
        w16[c, :64, 2 * c] = w_score
        w16[c, 64:, 2 * c + 1] = w_score
    w16 = np.concatenate(list(w16), axis=1)  # [128, 8*16]
    diagmask = np.zeros((H, DK), np.float32)
    for h in range(H):
        diagmask[h, h * 64:(h + 1) * 64] = 1.0
    qkb = (bq + bk).astype(np.float32)
    qkb4 = np.broadcast_to(qkb, (BL, DM)).copy()
    bf4 = np.broadcast_to(bf.astype(np.float32), (BL, DK)).copy()
    maskneg = (mask.astype(np.float32) * np.float32(-1e30))  # [B, LK]

    in_maps = []
    for i in range(NCORES):
        s = slice(i * BL, (i + 1) * BL)
        in_maps.append({
            "keyT": round_f32r(key[s].transpose(0, 2, 1)),
            "value": value[s],
            "query": query[s, 0, :],
            "maskneg": np.ascontiguousarray(
                np.broadcast_to(maskneg[s][:, None, :], (BL, H, LK))),
            "Wk": round_f32r(Wk),
            "Wq": round_f32r(Wq),
            "Wf": round_f32r(Wf),
            "w16": round_f32r(w16),
            "qkb4": qkb4,
            "bf4": bf4,
            "diagmask": diagmask,
        })
    return in_maps


def kernel(key, value, query, mask, Wk, bk, Wq, bq, w_score, b_score, Wf, bf):
    nc = _get_nc()
    in_maps = prep_in_maps(key, value, query, mask, Wk, bk, Wq, bq,
                           w_score, b_score, Wf, bf)
    res = run_bass_kernel_spmd(nc, in_maps, list(range(NCORES)))
    out = np.concatenate([res.results[i]["out"] for i in range(NCORES)], axis=0)
    attn = np.concatenate([res.results[i]["attn"] for i in range(NCORES)], axis=0)
    return out.reshape(B, 1, DK), attn.reshape(B, H, 1, LK)[:, :, :, :]


# revision 9
# speedup vs baseline: 1.0578x; 1.0578x over previous
"""Multi-head MLP (additive) attention on 8 TRN2 NeuronCores.

Data-parallel over batch: B=32 -> 4 batches per core. Each core computes
its shard fully (no collectives); host scatters inputs / gathers outputs.

Per-core pipeline (b = local batch, L=2048 keys, Dm=1024, H=16, d=64):
  key_up^T[c]  = Wk[:,c].T @ key^T          (f32r matmuls, PSUM accum)
  tanh_t       = tanh(key_up^T + (q@Wq+bq+bk)^T)   (ScalarE, fused bias)
  scores[2c]   = W2.T @ tanh_t              (per-head dot with w_score)
  attn         = softmax(scores + mask*-1e30)      (DVE/ACT, 16 partitions)
  ctx_full     = attn @ value               (wasteful [16,1024], diag-masked)
  out          = ctx_flat @ Wf + bf         (batched over 4 local b)

b_score is dropped: softmax is shift-invariant.
"""

import numpy as np

import concourse.bass as bass
import concourse.mybir as mybir
import concourse.tile as tile
from concourse import bacc
from concourse.bass_utils import run_bass_kernel_spmd
from concourse.masks import make_identity

F32 = mybir.dt.float32
F32R = mybir.dt.float32r
BF16 = mybir.dt.bfloat16
AF = mybir.ActivationFunctionType
ALU = mybir.AluOpType
AX = mybir.AxisListType

NCORES = 8
B, LK, DK, DM, H = 32, 2048, 1024, 1024, 16
BL = B // NCORES  # 4 local batches per core


def build_program():
    nc = bacc.Bacc("TRN2", target_bir_lowering=False, num_devices=NCORES)

    keyT_d = nc.dram_tensor("keyT", [BL, DK, LK], F32, kind="ExternalInput").ap()
    val_d = nc.dram_tensor("value", [BL, LK, DK], F32, kind="ExternalInput").ap()
    q_d = nc.dram_tensor("query", [BL, DM], F32, kind="ExternalInput").ap()
    mneg_d = nc.dram_tensor("maskneg", [BL, H, LK], F32, kind="ExternalInput").ap()
    wk_d = nc.dram_tensor("Wk", [DK, DM], F32, kind="ExternalInput").ap()
    wq_d = nc.dram_tensor("Wq", [DM, DM], F32R, kind="ExternalInput").ap()
    wf_d = nc.dram_tensor("Wf", [DK, DK], F32R, kind="ExternalInput").ap()
    w2_d = nc.dram_tensor("w16", [128, 8 * H], F32, kind="ExternalInput").ap()
    qkb_d = nc.dram_tensor("qkb4", [BL, DM], F32, kind="ExternalInput").ap()
    bf4_d = nc.dram_tensor("bf4", [BL, DK], F32, kind="ExternalInput").ap()
    diag_d = nc.dram_tensor("diagmask", [H, DK], F32, kind="ExternalInput").ap()

    out_d = nc.dram_tensor("out", [BL, DK], F32, kind="ExternalOutput").ap()
    attn_d = nc.dram_tensor("attn", [BL, H, LK], F32, kind="ExternalOutput").ap()

    with tile.TileContext(nc) as tc:
        _body(tc, nc, keyT_d, val_d, q_d, mneg_d, wk_d, wq_d, wf_d, w2_d,
              qkb_d, bf4_d, diag_d, out_d, attn_d)

    nc.compile()
    return nc


def _body(tc, nc, keyT_d, val_d, q_d, mneg_d, wk_d, wq_d, wf_d, w2_d,
          qkb_d, bf4_d, diag_d, out_d, attn_d):
    from contextlib import ExitStack

    ctx = ExitStack()
    const = ctx.enter_context(tc.tile_pool(name="const", bufs=1))
    bigw2 = ctx.enter_context(tc.tile_pool(name="bigw2", bufs=1))
    wkp = ctx.enter_context(tc.tile_pool(name="wkp", bufs=1))
    ktp = ctx.enter_context(tc.tile_pool(name="ktp", bufs=2))
    vp = ctx.enter_context(tc.tile_pool(name="vp", bufs=5))
    thp = ctx.enter_context(tc.tile_pool(name="thp", bufs=3))
    scp = ctx.enter_context(tc.tile_pool(name="scp", bufs=2))
    atp = ctx.enter_context(tc.tile_pool(name="atp", bufs=2))
    aTp = ctx.enter_context(tc.tile_pool(name="aTp", bufs=3))
    mkp = ctx.enter_context(tc.tile_pool(name="mkp", bufs=2))
    smp = ctx.enter_context(tc.tile_pool(name="smp", bufs=4))
    cmp_ = ctx.enter_context(tc.tile_pool(name="cmp", bufs=2))
    pku = ctx.enter_context(tc.tile_pool(name="pku", bufs=2, space="PSUM"))
    ptp = ctx.enter_context(tc.tile_pool(name="ptp", bufs=2, space="PSUM"))
    psc = ctx.enter_context(tc.tile_pool(name="psc", bufs=2, space="PSUM"))
    pcx = ctx.enter_context(tc.tile_pool(name="pcx", bufs=1, space="PSUM"))

    # ---- constants ----
    ident = const.tile([128, 128], F32)
    make_identity(nc, ident[:])
    w16_sb = const.tile([128, 8 * H], BF16)
    nc.gpsimd.dma_start(out=w16_sb[:], in_=w2_d[:])
    diag_sb = const.tile([H, DK], F32)
    nc.sync.dma_start(diag_sb[:], diag_d[:])
    qkb_sb = const.tile([BL, DM], F32)
    nc.sync.dma_start(qkb_sb[:], qkb_d[:])
    bf4_sb = const.tile([BL, DK], F32)
    nc.sync.dma_start(bf4_sb[:], bf4_d[:])
    qTb = const.tile([128, 8 * BL], F32)   # tanh bias, [128, c*4+b]
    ctxT = const.tile([128, 8 * BL], F32)  # ctx_flat^T, [128, c*4+b]

    # ---- query projection (prologue) ----
    wq_sb = bigw2.tile([128, 8 * DM], F32R, tag="bigw2", name="wq_sb")
    for k in range(8):
        nc.sync.dma_start(wq_sb[:, k * DM:(k + 1) * DM], wq_d[k * 128:(k + 1) * 128, :])
    q_sb = const.tile([BL, DM], F32)
    nc.sync.dma_start(q_sb[:], q_d[:])

    qT = const.tile([128, 8 * BL], F32R)
    for c in range(8):
        pt = ptp.tile([128, 16], F32, tag="pt", name="pt_q")
        nc.tensor.transpose(pt[:, :BL], q_sb[:, c * 128:(c + 1) * 128], ident[:BL, :BL])
        nc.any.tensor_copy(out=qT[:, c * BL:(c + 1) * BL], in_=pt[:, :BL])

    psq = pcx.tile([BL, DM], F32, tag="c", name="psq")
    for c in range(8):
        for hf in range(2):
            nc.tensor.matmul(
                psq[:, hf * 512:(hf + 1) * 512],
                qT[:, c * BL:(c + 1) * BL],
                wq_sb[:, c * DM + hf * 512: c * DM + (hf + 1) * 512],
                start=(c == 0), stop=(c == 7),
            )
    qsum = const.tile([BL, DM], F32)
    nc.vector.tensor_tensor(qsum[:], psq[:], qkb_sb[:], ALU.add)
    for c in range(8):
        pt = ptp.tile([128, 16], F32, tag="pt", name="pt_qb")
        nc.tensor.transpose(pt[:, :BL], qsum[:, c * 128:(c + 1) * 128], ident[:BL, :BL])
        nc.any.tensor_copy(out=qTb[:, c * BL:(c + 1) * BL], in_=pt[:, :BL])

    # ---- Wk ----
    wk_sb = wkp.tile([128, 8 * DM], BF16)
    for k in range(8):
        nc.gpsimd.dma_start(out=wk_sb[:, k * DM:(k + 1) * DM],
                            in_=wk_d[k * 128:(k + 1) * 128, :])

    # ---- main loop over local batches ----
    for b in range(BL):
        mneg = mkp.tile([H, LK], F32)
        nc.sync.dma_start(mneg[:], mneg_d[b])
        scores = scp.tile([H, LK], F32)

        for j in range(4):  # L chunks of 512
            kT = ktp.tile([128, 8 * 512], BF16, tag="kT")
            for k in range(8):
                nc.gpsimd.dma_start(
                    out=kT[:, k * 512:(k + 1) * 512],
                    in_=keyT_d[b, k * 128:(k + 1) * 128, j * 512:(j + 1) * 512],
                )
            ps = psc.tile([H, 512], F32, tag="s")
            for c in range(8):
                pk = pku.tile([128, 512], F32, tag="t", name="pk")
                for k in range(8):
                    nc.tensor.matmul(
                        pk[:],
                        wk_sb[:, k * DM + c * 128: k * DM + (c + 1) * 128],
                        kT[:, k * 512:(k + 1) * 512],
                        start=(k == 0), stop=(k == 7),
                    )
                th = thp.tile([128, 512], BF16)
                nc.scalar.activation(
                    th[:], pk[:], AF.Tanh,
                    bias=qTb[:, c * BL + b: c * BL + b + 1], scale=1.0,
                )
                nc.tensor.matmul(
                    ps[:], w16_sb[:, c * H:(c + 1) * H], th[:],
                    start=(c == 0), stop=(c == 7),
                )
            nc.vector.tensor_tensor(
                scores[:, j * 512:(j + 1) * 512], ps[:],
                mneg[:, j * 512:(j + 1) * 512], ALU.add,
            )

        # softmax over L (free axis), 16 head-partitions
        negmax = smp.tile([H, 1], F32)
        nc.vector.reduce_max(negmax[:], scores[:], axis=AX.X, negate=True)
        attn = atp.tile([H, LK], F32)
        sums = smp.tile([H, 1], F32)
        nc.scalar.activation(attn[:], scores[:], AF.Exp, bias=negmax[:],
                             accum_out=sums[:])
        recip = smp.tile([H, 1], F32)
        nc.vector.reciprocal(recip[:], sums[:])
        nc.vector.tensor_scalar_mul(attn[:], attn[:], recip[:])
        nc.sync.dma_start(attn_d[b], attn[:])

        # context: ctx_full[16,1024] = sum_j2 attnT[j2].T @ value[j2]
        pc = pcx.tile([H, DK], F32, tag="c", name="pc")
        for j2 in range(16):
            pt = ptp.tile([128, 16], F32, tag="pt", name="pt_a")
            nc.tensor.transpose(
                pt[:, :H], attn[:, j2 * 128:(j2 + 1) * 128], ident[:H, :H])
            aT = aTp.tile([128, H], BF16)
            nc.any.tensor_copy(out=aT[:], in_=pt[:, :H])
            v = vp.tile([128, DK], BF16)
            nc.gpsimd.dma_start(out=v[:], in_=val_d[b, j2 * 128:(j2 + 1) * 128, :])
            for hf in range(2):
                nc.tensor.matmul(
                    pc[:, hf * 512:(hf + 1) * 512], aT[:],
                    v[:, hf * 512:(hf + 1) * 512],
                    start=(j2 == 0), stop=(j2 == 15),
                )
        ctxm = cmp_.tile([H, DK], F32)
        nc.vector.tensor_tensor(ctxm[:], pc[:], diag_sb[:], ALU.mult)
        for c in range(8):
            pt = ptp.tile([128, 16], F32, tag="pt", name="pt_c")
            nc.tensor.transpose(
                pt[:, :H], ctxm[:, c * 128:(c + 1) * 128], ident[:H, :H])
            nc.vector.reduce_sum(
                ctxT[:, c * BL + b: c * BL + b + 1], pt[:, :H], axis=AX.X)

    # rounded copy of ctxT for the final f32r matmuls
    ctxTr = const.tile([128, 8 * BL], F32R)
    nc.any.tensor_copy(out=ctxTr[:], in_=ctxT[:])

    # ---- final projection: out[4,1024] = ctx_flat @ Wf + bf ----
    wf_sb = bigw2.tile([128, 8 * DK], F32R, tag="bigw2", name="wf_sb")
    for k in range(8):
        nc.sync.dma_start(wf_sb[:, k * DK:(k + 1) * DK], wf_d[k * 128:(k + 1) * 128, :])
    po = pcx.tile([BL, DK], F32, tag="c", name="po")
    for c in range(8):
        for hf in range(2):
            nc.tensor.matmul(
                po[:, hf * 512:(hf + 1) * 512],
                ctxTr[:, c * BL:(c + 1) * BL],
                wf_sb[:, c * DK + hf * 512: c * DK + (hf + 1) * 512],
                start=(c == 0), stop=(c == 7),
            )
    out_sb = const.tile([BL, DK], F32)
    nc.vector.tensor_tensor(out_sb[:], po[:], bf4_sb[:], ALU.add)
    nc.sync.dma_start(out_d[:], out_sb[:])

    ctx.close()


_CACHED_NC = None


def _get_nc():
    global _CACHED_NC
    if _CACHED_NC is None:
        _CACHED_NC = build_program()
    return _CACHED_NC


def round_f32r(x):
    u = np.ascontiguousarray(x, np.float32).view(np.uint32)
    u = (u + np.uint32(0x800)) & np.uint32(0xFFFFF000)
    return u.view(np.float32)


def prep_in_maps(key, value, query, mask, Wk, bk, Wq, bq, w_score, b_score,
                 Wf, bf):
    key = np.ascontiguousarray(key, dtype=np.float32)
    value = np.ascontiguousarray(value, dtype=np.float32)
    query = np.ascontiguousarray(query, dtype=np.float32)

    w16 = np.zeros((8, 128, H), np.float32)
    for c in range(8):
        w16[c, :64, 2 * c] = w_score
        w16[c, 64:, 2 * c + 1] = w_score
    w16 = np.concatenate(list(w16), axis=1)  # [128, 8*16]
    diagmask = np.zeros((H, DK), np.float32)
    for h in range(H):
        diagmask[h, h * 64:(h + 1) * 64] = 1.0
    qkb = (bq + bk).astype(np.float32)
    qkb4 = np.broadcast_to(qkb, (BL, DM)).copy()
    bf4 = np.broadcast_to(bf.astype(np.float32), (BL, DK)).copy()
    maskneg = (mask.astype(np.float32) * np.float32(-1e30))  # [B, LK]

    in_maps = []
    for i in range(NCORES):
        s = slice(i * BL, (i + 1) * BL)
        in_maps.append({
            "keyT": np.ascontiguousarray(key[s].transpose(0, 2, 1)),
            "value": value[s],
            "query": query[s, 0, :],
            "maskneg": np.ascontiguousarray(
                np.broadcast_to(maskneg[s][:, None, :], (BL, H, LK))),
            "Wk": Wk.astype(np.float32),
            "Wq": round_f32r(Wq),
            "Wf": round_f32r(Wf),
            "w16": w16,
            "qkb4": qkb4,
            "bf4": bf4,
            "diagmask": diagmask,
        })
    return in_maps


def kernel(key, value, query, mask, Wk, bk, Wq, bq, w_score, b_score, Wf, bf):
    nc = _get_nc()
    in_maps = prep_in_maps(key, value, query, mask, Wk, bk, Wq, bq,
                           w_score, b_score, Wf, bf)
    res = run_bass_kernel_spmd(nc, in_maps, list(range(NCORES)))
    out = np.concatenate([res.results[i]["out"] for i in range(NCORES)], axis=0)
    attn = np.concatenate([res.results[i]["attn"] for i in range(NCORES)], axis=0)
    return out.reshape(B, 1, DK), attn.reshape(B, H, 1, LK)[:, :, :, :]


# revision 10
# speedup vs baseline: 1.0633x; 1.0052x over previous
"""Multi-head MLP (additive) attention on 8 TRN2 NeuronCores.

Data-parallel over batch: B=32 -> 4 batches per core. Each core computes
its shard fully (no collectives); host scatters inputs / gathers outputs.

Per-core pipeline (b = local batch, L=2048 keys, Dm=1024, H=16, d=64):
  key_up^T[c]  = Wk[:,c].T @ key^T          (f32r matmuls, PSUM accum)
  tanh_t       = tanh(key_up^T + (q@Wq+bq+bk)^T)   (ScalarE, fused bias)
  scores[2c]   = W2.T @ tanh_t              (per-head dot with w_score)
  attn         = softmax(scores + mask*-1e30)      (DVE/ACT, 16 partitions)
  ctx_full     = attn @ value               (wasteful [16,1024], diag-masked)
  out          = ctx_flat @ Wf + bf         (batched over 4 local b)

b_score is dropped: softmax is shift-invariant.
"""

import numpy as np

import concourse.bass as bass
import concourse.mybir as mybir
import concourse.tile as tile
from concourse import bacc
from concourse.bass_utils import run_bass_kernel_spmd
from concourse.masks import make_identity

F32 = mybir.dt.float32
F32R = mybir.dt.float32r
BF16 = mybir.dt.bfloat16
AF = mybir.ActivationFunctionType
ALU = mybir.AluOpType
AX = mybir.AxisListType

NCORES = 8
B, LK, DK, DM, H = 32, 2048, 1024, 1024, 16
BL = B // NCORES  # 4 local batches per core


def build_program():
    nc = bacc.Bacc("TRN2", target_bir_lowering=False, num_devices=NCORES,
                   num_swdge_queues=4)

    keyT_d = nc.dram_tensor("keyT", [BL, DK, LK], F32, kind="ExternalInput").ap()
    val_d = nc.dram_tensor("value", [BL, LK, DK], F32, kind="ExternalInput").ap()
    q_d = nc.dram_tensor("query", [BL, DM], F32, kind="ExternalInput").ap()
    mneg_d = nc.dram_tensor("maskneg", [BL, H, LK], F32, kind="ExternalInput").ap()
    wk_d = nc.dram_tensor("Wk", [DK, DM], F32, kind="ExternalInput").ap()
    wq_d = nc.dram_tensor("Wq", [DM, DM], F32R, kind="ExternalInput").ap()
    wf_d = nc.dram_tensor("Wf", [DK, DK], F32R, kind="ExternalInput").ap()
    w2_d = nc.dram_tensor("w16", [128, 8 * H], F32, kind="ExternalInput").ap()
    qkb_d = nc.dram_tensor("qkb4", [BL, DM], F32, kind="ExternalInput").ap()
    bf4_d = nc.dram_tensor("bf4", [BL, DK], F32, kind="ExternalInput").ap()
    diag_d = nc.dram_tensor("diagmask", [H, DK], F32, kind="ExternalInput").ap()

    out_d = nc.dram_tensor("out", [BL, DK], F32, kind="ExternalOutput").ap()
    attn_d = nc.dram_tensor("attn", [BL, H, LK], F32, kind="ExternalOutput").ap()

    with tile.TileContext(nc) as tc:
        _body(tc, nc, keyT_d, val_d, q_d, mneg_d, wk_d, wq_d, wf_d, w2_d,
              qkb_d, bf4_d, diag_d, out_d, attn_d)

    nc.compile()
    return nc


def _body(tc, nc, keyT_d, val_d, q_d, mneg_d, wk_d, wq_d, wf_d, w2_d,
          qkb_d, bf4_d, diag_d, out_d, attn_d):
    from contextlib import ExitStack

    ctx = ExitStack()
    const = ctx.enter_context(tc.tile_pool(name="const", bufs=1))
    bigw2 = ctx.enter_context(tc.tile_pool(name="bigw2", bufs=1))
    wkp = ctx.enter_context(tc.tile_pool(name="wkp", bufs=1))
    ktp = ctx.enter_context(tc.tile_pool(name="ktp", bufs=3))
    vp = ctx.enter_context(tc.tile_pool(name="vp", bufs=18))
    thp = ctx.enter_context(tc.tile_pool(name="thp", bufs=3))
    scp = ctx.enter_context(tc.tile_pool(name="scp", bufs=2))
    atp = ctx.enter_context(tc.tile_pool(name="atp", bufs=2))
    aTp = ctx.enter_context(tc.tile_pool(name="aTp", bufs=3))
    mkp = ctx.enter_context(tc.tile_pool(name="mkp", bufs=2))
    smp = ctx.enter_context(tc.tile_pool(name="smp", bufs=4))
    cmp_ = ctx.enter_context(tc.tile_pool(name="cmp", bufs=2))
    pku = ctx.enter_context(tc.tile_pool(name="pku", bufs=3, space="PSUM"))
    ptp = ctx.enter_context(tc.tile_pool(name="ptp", bufs=1, space="PSUM"))
    psc = ctx.enter_context(tc.tile_pool(name="psc", bufs=2, space="PSUM"))
    pcx = ctx.enter_context(tc.tile_pool(name="pcx", bufs=1, space="PSUM"))

    # ---- constants ----
    ident = const.tile([128, 128], F32)
    make_identity(nc, ident[:])
    w16_sb = const.tile([128, 8 * H], BF16)
    nc.gpsimd.dma_start(out=w16_sb[:], in_=w2_d[:])
    diag_sb = const.tile([H, DK], F32)
    nc.sync.dma_start(diag_sb[:], diag_d[:])
    qkb_sb = const.tile([BL, DM], F32)
    nc.sync.dma_start(qkb_sb[:], qkb_d[:])
    bf4_sb = const.tile([BL, DK], F32)
    nc.sync.dma_start(bf4_sb[:], bf4_d[:])
    qTb = const.tile([128, 8 * BL], F32)   # tanh bias, [128, c*4+b]
    ctxT = const.tile([128, 8 * BL], F32)  # ctx_flat^T, [128, c*4+b]

    # ---- query projection (prologue) ----
    wq_sb = bigw2.tile([128, 8 * DM], F32R, tag="bigw2", name="wq_sb")
    for k in range(8):
        nc.sync.dma_start(wq_sb[:, k * DM:(k + 1) * DM], wq_d[k * 128:(k + 1) * 128, :])
    q_sb = const.tile([BL, DM], F32)
    nc.sync.dma_start(q_sb[:], q_d[:])

    qT = const.tile([128, 8 * BL], F32R)
    for c in range(8):
        pt = ptp.tile([128, 16], F32, tag="pt", name="pt_q")
        nc.tensor.transpose(pt[:, :BL], q_sb[:, c * 128:(c + 1) * 128], ident[:BL, :BL])
        nc.any.tensor_copy(out=qT[:, c * BL:(c + 1) * BL], in_=pt[:, :BL])

    psq = pcx.tile([BL, DM], F32, tag="c", name="psq")
    for c in range(8):
        for hf in range(2):
            nc.tensor.matmul(
                psq[:, hf * 512:(hf + 1) * 512],
                qT[:, c * BL:(c + 1) * BL],
                wq_sb[:, c * DM + hf * 512: c * DM + (hf + 1) * 512],
                start=(c == 0), stop=(c == 7),
            )
    qsum = const.tile([BL, DM], F32)
    nc.vector.tensor_tensor(qsum[:], psq[:], qkb_sb[:], ALU.add)
    for c in range(8):
        pt = ptp.tile([128, 16], F32, tag="pt", name="pt_qb")
        nc.tensor.transpose(pt[:, :BL], qsum[:, c * 128:(c + 1) * 128], ident[:BL, :BL])
        nc.any.tensor_copy(out=qTb[:, c * BL:(c + 1) * BL], in_=pt[:, :BL])

    # ---- Wk ----
    wk_sb = wkp.tile([128, 8 * DM], BF16)
    for k in range(8):
        nc.gpsimd.dma_start(out=wk_sb[:, k * DM:(k + 1) * DM],
                            in_=wk_d[k * 128:(k + 1) * 128, :])

    # ---- main loop over local batches ----
    for b in range(BL):
        mneg = mkp.tile([H, LK], F32)
        nc.sync.dma_start(mneg[:], mneg_d[b])
        scores = scp.tile([H, LK], F32)

        vts = []
        for j2 in range(16):
            v = vp.tile([128, DK], BF16, tag="v", name="v")
            nc.gpsimd.dma_start(out=v[:], in_=val_d[b, j2 * 128:(j2 + 1) * 128, :])
            vts.append(v)

        for j in range(4):  # L chunks of 512
            kT = ktp.tile([128, 8 * 512], BF16, tag="kT")
            for k in range(8):
                nc.gpsimd.dma_start(
                    out=kT[:, k * 512:(k + 1) * 512],
                    in_=keyT_d[b, k * 128:(k + 1) * 128, j * 512:(j + 1) * 512],
                )
            ps = psc.tile([H, 512], F32, tag="s")
            for c in range(8):
                pk = pku.tile([128, 512], F32, tag="t", name="pk")
                for k in range(8):
                    nc.tensor.matmul(
                        pk[:],
                        wk_sb[:, k * DM + c * 128: k * DM + (c + 1) * 128],
                        kT[:, k * 512:(k + 1) * 512],
                        start=(k == 0), stop=(k == 7),
                    )
                th = thp.tile([128, 512], BF16)
                nc.scalar.activation(
                    th[:], pk[:], AF.Tanh,
                    bias=qTb[:, c * BL + b: c * BL + b + 1], scale=1.0,
                )
                nc.tensor.matmul(
                    ps[:], w16_sb[:, c * H:(c + 1) * H], th[:],
                    start=(c == 0), stop=(c == 7),
                )
            nc.vector.tensor_tensor(
                scores[:, j * 512:(j + 1) * 512], ps[:],
                mneg[:, j * 512:(j + 1) * 512], ALU.add,
            )

        # softmax over L (free axis), 16 head-partitions
        negmax = smp.tile([H, 1], F32)
        nc.vector.reduce_max(negmax[:], scores[:], axis=AX.X, negate=True)
        attn = atp.tile([H, LK], F32)
        sums = smp.tile([H, 1], F32)
        nc.scalar.activation(attn[:], scores[:], AF.Exp, bias=negmax[:],
                             accum_out=sums[:])
        recip = smp.tile([H, 1], F32)
        nc.vector.reciprocal(recip[:], sums[:])
        nc.vector.tensor_scalar_mul(attn[:], attn[:], recip[:])
        nc.sync.dma_start(attn_d[b], attn[:])

        # context: ctx_full[16,1024] = sum_j2 attnT[j2].T @ value[j2]
        pc = pcx.tile([H, DK], F32, tag="c", name="pc")
        for j2 in range(16):
            pt = ptp.tile([128, 16], F32, tag="pt", name="pt_a")
            nc.tensor.transpose(
                pt[:, :H], attn[:, j2 * 128:(j2 + 1) * 128], ident[:H, :H])
            aT = aTp.tile([128, H], BF16)
            nc.any.tensor_copy(out=aT[:], in_=pt[:, :H])
            v = vts[j2]
            for hf in range(2):
                nc.tensor.matmul(
                    pc[:, hf * 512:(hf + 1) * 512], aT[:],
                    v[:, hf * 512:(hf + 1) * 512],
                    start=(j2 == 0), stop=(j2 == 15),
                )
        ctxm = cmp_.tile([H, DK], F32)
        nc.vector.tensor_tensor(ctxm[:], pc[:], diag_sb[:], ALU.mult)
        for c in range(8):
            pt = ptp.tile([128, 16], F32, tag="pt", name="pt_c")
            nc.tensor.transpose(
                pt[:, :H], ctxm[:, c * 128:(c + 1) * 128], ident[:H, :H])
            nc.vector.reduce_sum(
                ctxT[:, c * BL + b: c * BL + b + 1], pt[:, :H], axis=AX.X)

    # rounded copy of ctxT for the final f32r matmuls
    ctxTr = const.tile([128, 8 * BL], F32R)
    nc.any.tensor_copy(out=ctxTr[:], in_=ctxT[:])

    # ---- final projection: out[4,1024] = ctx_flat @ Wf + bf ----
    wf_sb = bigw2.tile([128, 8 * DK], F32R, tag="bigw2", name="wf_sb")
    for k in range(8):
        nc.sync.dma_start(wf_sb[:, k * DK:(k + 1) * DK], wf_d[k * 128:(k + 1) * 128, :])
    po = pcx.tile([BL, DK], F32, tag="c", name="po")
    for c in range(8):
        for hf in range(2):
            nc.tensor.matmul(
                po[:, hf * 512:(hf + 1) * 512],
                ctxTr[:, c * BL:(c + 1) * BL],
                wf_sb[:, c * DK + hf * 512: c * DK + (hf + 1) * 512],
                start=(c == 0), stop=(c == 7),
            )
    out_sb = const.tile([BL, DK], F32)
    nc.vector.tensor_tensor(out_sb[:], po[:], bf4_sb[:], ALU.add)
    nc.sync.dma_start(out_d[:], out_sb[:])

    ctx.close()


_CACHED_NC = None


def _get_nc():
    global _CACHED_NC
    if _CACHED_NC is None:
        _CACHED_NC = build_program()
    return _CACHED_NC


def round_f32r(x):
    u = np.ascontiguousarray(x, np.float32).view(np.uint32)
    u = (u + np.uint32(0x800)) & np.uint32(0xFFFFF000)
    return u.view(np.float32)


def prep_in_maps(key, value, query, mask, Wk, bk, Wq, bq, w_score, b_score,
                 Wf, bf):
    key = np.ascontiguousarray(key, dtype=np.float32)
    value = np.ascontiguousarray(value, dtype=np.float32)
    query = np.ascontiguousarray(query, dtype=np.float32)

    w16 = np.zeros((8, 128, H), np.float32)
    for c in range(8):
        w16[c, :64, 2 * c] = w_score
        w16[c, 64:, 2 * c + 1] = w_score
    w16 = np.concatenate(list(w16), axis=1)  # [128, 8*16]
    diagmask = np.zeros((H, DK), np.float32)
    for h in range(H):
        diagmask[h, h * 64:(h + 1) * 64] = 1.0
    qkb = (bq + bk).astype(np.float32)
    qkb4 = np.broadcast_to(qkb, (BL, DM)).copy()
    bf4 = np.broadcast_to(bf.astype(np.float32), (BL, DK)).copy()
    maskneg = (mask.astype(np.float32) * np.float32(-1e30))  # [B, LK]

    in_maps = []
    for i in range(NCORES):
        s = slice(i * BL, (i + 1) * BL)
        in_maps.append({
            "keyT": np.ascontiguousarray(key[s].transpose(0, 2, 1)),
            "value": value[s],
            "query": query[s, 0, :],
            "maskneg": np.ascontiguousarray(
                np.broadcast_to(maskneg[s][:, None, :], (BL, H, LK))),
            "Wk": Wk.astype(np.float32),
            "Wq": round_f32r(Wq),
            "Wf": round_f32r(Wf),
            "w16": w16,
            "qkb4": qkb4,
            "bf4": bf4,
            "diagmask": diagmask,
        })
    return in_maps


def kernel(key, value, query, mask, Wk, bk, Wq, bq, w_score, b_score, Wf, bf):
    nc = _get_nc()
    in_maps = prep_in_maps(key, value, query, mask, Wk, bk, Wq, bq,
                           w_score, b_score, Wf, bf)
    res = run_bass_kernel_spmd(nc, in_maps, list(range(NCORES)))
    out = np.concatenate([res.results[i]["out"] for i in range(NCORES)], axis=0)
    attn = np.concatenate([res.results[i]["attn"] for i in range(NCORES)], axis=0)
    return out.reshape(B, 1, DK), attn.reshape(B, H, 1, LK)[:, :, :, :]


# revision 11
# speedup vs baseline: 1.1857x; 1.1152x over previous
"""Multi-head MLP (additive) attention on 8 TRN2 NeuronCores.

Data-parallel over batch: B=32 -> 4 batches per core. Each core computes
its shard fully (no collectives); host scatters inputs / gathers outputs.

Host prep: key is transposed to [B, Dk, Lk] and all matmul operands are
converted to bf16 (halves DMA bytes, full-rate PE streaming).

Per-core pipeline (b = local batch, L=2048 keys, Dm=1024, H=16, d=64):
  key_up^T[c]  = Wk[:,c].T @ key^T                 (bf16 matmuls, PSUM f32)
  tanh_t       = tanh(key_up^T + (q@Wq+bq+bk)^T)   (ScalarE, fused bias)
  scores       = W16c.T @ tanh_t  (accum over c)   (per-head dot, 16 rows)
  attn         = softmax(scores + mask*-1e30)      (DVE/ACT, 16 partitions)
  ctx_full     = attn @ value                      ([16,1024], diag-masked)
  out          = ctx_flat @ Wf + bf                (batched over 4 local b)

b_score is dropped: softmax is shift-invariant.
"""

import ml_dtypes
import numpy as np

import concourse.bass as bass
import concourse.mybir as mybir
import concourse.tile as tile
from concourse import bacc
from concourse.bass_utils import run_bass_kernel_spmd
from concourse.masks import make_identity

F32 = mybir.dt.float32
BF16 = mybir.dt.bfloat16
AF = mybir.ActivationFunctionType
ALU = mybir.AluOpType
AX = mybir.AxisListType

NCORES = 8
B, LK, DK, DM, H = 32, 2048, 1024, 1024, 16
BL = B // NCORES  # 4 local batches per core


def build_program():
    nc = bacc.Bacc("TRN2", target_bir_lowering=False, num_devices=NCORES)

    keyT_d = nc.dram_tensor("keyT", [BL, DK, LK], BF16, kind="ExternalInput").ap()
    val_d = nc.dram_tensor("value", [BL, LK, DK], BF16, kind="ExternalInput").ap()
    q_d = nc.dram_tensor("query", [BL, DM], F32, kind="ExternalInput").ap()
    mneg_d = nc.dram_tensor("maskneg", [BL, H, LK], F32, kind="ExternalInput").ap()
    wk_d = nc.dram_tensor("Wk", [DK, DM], BF16, kind="ExternalInput").ap()
    wq_d = nc.dram_tensor("Wq", [DM, DM], BF16, kind="ExternalInput").ap()
    wf_d = nc.dram_tensor("Wf", [DK, DK], BF16, kind="ExternalInput").ap()
    w16_d = nc.dram_tensor("w16", [128, 8 * H], BF16, kind="ExternalInput").ap()
    qkb_d = nc.dram_tensor("qkb4", [BL, DM], F32, kind="ExternalInput").ap()
    bf4_d = nc.dram_tensor("bf4", [BL, DK], F32, kind="ExternalInput").ap()
    diag_d = nc.dram_tensor("diagmask", [H, DK], F32, kind="ExternalInput").ap()

    out_d = nc.dram_tensor("out", [BL, DK], F32, kind="ExternalOutput").ap()
    attn_d = nc.dram_tensor("attn", [BL, H, LK], F32, kind="ExternalOutput").ap()

    with tile.TileContext(nc) as tc:
        _body(tc, nc, keyT_d, val_d, q_d, mneg_d, wk_d, wq_d, wf_d, w16_d,
              qkb_d, bf4_d, diag_d, out_d, attn_d)

    nc.compile()
    return nc


def _body(tc, nc, keyT_d, val_d, q_d, mneg_d, wk_d, wq_d, wf_d, w16_d,
          qkb_d, bf4_d, diag_d, out_d, attn_d):
    from contextlib import ExitStack

    ctx = ExitStack()
    const = ctx.enter_context(tc.tile_pool(name="const", bufs=1))
    bigw2 = ctx.enter_context(tc.tile_pool(name="bigw2", bufs=1))
    wkp = ctx.enter_context(tc.tile_pool(name="wkp", bufs=1))
    ktp = ctx.enter_context(tc.tile_pool(name="ktp", bufs=3))
    vp = ctx.enter_context(tc.tile_pool(name="vp", bufs=18))
    thp = ctx.enter_context(tc.tile_pool(name="thp", bufs=3))
    scp = ctx.enter_context(tc.tile_pool(name="scp", bufs=2))
    atp = ctx.enter_context(tc.tile_pool(name="atp", bufs=2))
    aTp = ctx.enter_context(tc.tile_pool(name="aTp", bufs=3))
    mkp = ctx.enter_context(tc.tile_pool(name="mkp", bufs=2))
    smp = ctx.enter_context(tc.tile_pool(name="smp", bufs=4))
    cmp_ = ctx.enter_context(tc.tile_pool(name="cmp", bufs=2))
    pku = ctx.enter_context(tc.tile_pool(name="pku", bufs=3, space="PSUM"))
    ptp = ctx.enter_context(tc.tile_pool(name="ptp", bufs=2, space="PSUM"))
    psc = ctx.enter_context(tc.tile_pool(name="psc", bufs=1, space="PSUM"))
    pcx = ctx.enter_context(tc.tile_pool(name="pcx", bufs=1, space="PSUM"))

    # ---- constants ----
    ident = const.tile([128, 128], F32)
    make_identity(nc, ident[:])
    w16_sb = const.tile([128, 8 * H], BF16)
    nc.sync.dma_start(w16_sb[:], w16_d[:])
    diag_sb = const.tile([H, DK], F32)
    nc.sync.dma_start(diag_sb[:], diag_d[:])
    qkb_sb = const.tile([BL, DM], F32)
    nc.sync.dma_start(qkb_sb[:], qkb_d[:])
    bf4_sb = const.tile([BL, DK], F32)
    nc.sync.dma_start(bf4_sb[:], bf4_d[:])
    qTb = const.tile([128, 8 * BL], F32)    # tanh bias, [128, c*4+b]
    ctxT = const.tile([128, 8 * BL], F32)   # ctx_flat^T accum, f32
    ctxTr = const.tile([128, 8 * BL], BF16)  # rounded for final matmul

    # ---- query projection (prologue; must finish before first tanh) ----
    wq_sb = bigw2.tile([128, 8 * DM], BF16, tag="bigw2", name="wq_sb")
    for k in range(8):
        nc.sync.dma_start(wq_sb[:, k * DM:(k + 1) * DM], wq_d[k * 128:(k + 1) * 128, :])
    q_sb = const.tile([BL, DM], F32)
    nc.sync.dma_start(q_sb[:], q_d[:])

    qT = const.tile([128, 8 * BL], BF16)
    for c in range(8):
        pt = ptp.tile([128, 16], F32, tag="pt", name="pt_q")
        nc.tensor.transpose(pt[:, :BL], q_sb[:, c * 128:(c + 1) * 128], ident[:BL, :BL])
        nc.vector.tensor_copy(out=qT[:, c * BL:(c + 1) * BL], in_=pt[:, :BL])

    psq = pcx.tile([BL, DM], F32, tag="c", name="psq")
    for c in range(8):
        for hf in range(2):
            nc.tensor.matmul(
                psq[:, hf * 512:(hf + 1) * 512],
                qT[:, c * BL:(c + 1) * BL],
                wq_sb[:, c * DM + hf * 512: c * DM + (hf + 1) * 512],
                start=(c == 0), stop=(c == 7),
            )
    qsum = const.tile([BL, DM], F32)
    nc.vector.tensor_tensor(qsum[:], psq[:], qkb_sb[:], ALU.add)
    for c in range(8):
        pt = ptp.tile([128, 16], F32, tag="pt", name="pt_qb")
        nc.tensor.transpose(pt[:, :BL], qsum[:, c * 128:(c + 1) * 128], ident[:BL, :BL])
        nc.vector.tensor_copy(out=qTb[:, c * BL:(c + 1) * BL], in_=pt[:, :BL])

    # ---- Wk ----
    wk_sb = wkp.tile([128, 8 * DM], BF16)
    for k in range(8):
        nc.sync.dma_start(wk_sb[:, k * DM:(k + 1) * DM], wk_d[k * 128:(k + 1) * 128, :])

    # ---- main loop over local batches ----
    for b in range(BL):
        mneg = mkp.tile([H, LK], F32)
        nc.sync.dma_start(mneg[:], mneg_d[b])
        scores = scp.tile([H, LK], F32)

        vts = []
        for j2 in range(16):
            v = vp.tile([128, DK], BF16, tag="v", name="v")
            nc.sync.dma_start(v[:], val_d[b, j2 * 128:(j2 + 1) * 128, :])
            vts.append(v)

        for j in range(4):  # L chunks of 512
            kT = ktp.tile([128, 8 * 512], BF16, tag="kT")
            for k in range(8):
                nc.sync.dma_start(
                    kT[:, k * 512:(k + 1) * 512],
                    keyT_d[b, k * 128:(k + 1) * 128, j * 512:(j + 1) * 512],
                )
            ps = psc.tile([H, 512], F32, tag="s")
            for c in range(8):
                pk = pku.tile([128, 512], F32, tag="t", name="pk")
                for k in range(8):
                    nc.tensor.matmul(
                        pk[:],
                        wk_sb[:, k * DM + c * 128: k * DM + (c + 1) * 128],
                        kT[:, k * 512:(k + 1) * 512],
                        start=(k == 0), stop=(k == 7),
                    )
                th = thp.tile([128, 512], BF16)
                nc.scalar.activation(
                    th[:], pk[:], AF.Tanh,
                    bias=qTb[:, c * BL + b: c * BL + b + 1], scale=1.0,
                )
                nc.tensor.matmul(
                    ps[:], w16_sb[:, c * H:(c + 1) * H], th[:],
                    start=(c == 0), stop=(c == 7),
                )
            nc.vector.tensor_tensor(
                scores[:, j * 512:(j + 1) * 512], ps[:],
                mneg[:, j * 512:(j + 1) * 512], ALU.add,
            )

        # softmax over L (free axis), 16 head-partitions
        negmax = smp.tile([H, 1], F32)
        nc.vector.reduce_max(negmax[:], scores[:], axis=AX.X, negate=True)
        attn = atp.tile([H, LK], F32)
        sums = smp.tile([H, 1], F32)
        nc.scalar.activation(attn[:], scores[:], AF.Exp, bias=negmax[:],
                             accum_out=sums[:])
        recip = smp.tile([H, 1], F32)
        nc.vector.reciprocal(recip[:], sums[:])
        nc.vector.tensor_scalar_mul(attn[:], attn[:], recip[:])
        nc.sync.dma_start(attn_d[b], attn[:])

        # context: ctx_full[16,1024] = sum_j2 attnT[j2].T @ value[j2]
        pc = pcx.tile([H, DK], F32, tag="c", name="pc")
        for j2 in range(16):
            pt = ptp.tile([128, 16], F32, tag="pt", name="pt_a")
            nc.tensor.transpose(
                pt[:, :H], attn[:, j2 * 128:(j2 + 1) * 128], ident[:H, :H])
            aT = aTp.tile([128, H], BF16)
            nc.vector.tensor_copy(out=aT[:], in_=pt[:, :H])
            v = vts[j2]
            for hf in range(2):
                nc.tensor.matmul(
                    pc[:, hf * 512:(hf + 1) * 512], aT[:],
                    v[:, hf * 512:(hf + 1) * 512],
                    start=(j2 == 0), stop=(j2 == 15),
                )
        ctxm = cmp_.tile([H, DK], F32)
        nc.vector.tensor_tensor(ctxm[:], pc[:], diag_sb[:], ALU.mult)
        for c in range(8):
            pt = ptp.tile([128, 16], F32, tag="pt", name="pt_c")
            nc.tensor.transpose(
                pt[:, :H], ctxm[:, c * 128:(c + 1) * 128], ident[:H, :H])
            nc.vector.reduce_sum(
                ctxT[:, c * BL + b: c * BL + b + 1], pt[:, :H], axis=AX.X)

    # ---- final projection: out[4,1024] = ctx_flat @ Wf + bf ----
    nc.vector.tensor_copy(out=ctxTr[:], in_=ctxT[:])
    wf_sb = bigw2.tile([128, 8 * DK], BF16, tag="bigw2", name="wf_sb")
    for k in range(8):
        nc.sync.dma_start(wf_sb[:, k * DK:(k + 1) * DK], wf_d[k * 128:(k + 1) * 128, :])
    po = pcx.tile([BL, DK], F32, tag="c", name="po")
    for c in range(8):
        for hf in range(2):
            nc.tensor.matmul(
                po[:, hf * 512:(hf + 1) * 512],
                ctxTr[:, c * BL:(c + 1) * BL],
                wf_sb[:, c * DK + hf * 512: c * DK + (hf + 1) * 512],
                start=(c == 0), stop=(c == 7),
            )
    out_sb = const.tile([BL, DK], F32)
    nc.vector.tensor_tensor(out_sb[:], po[:], bf4_sb[:], ALU.add)
    nc.sync.dma_start(out_d[:], out_sb[:])

    ctx.close()


_CACHED_NC = None


def _get_nc():
    global _CACHED_NC
    if _CACHED_NC is None:
        _CACHED_NC = build_program()
    return _CACHED_NC


def _bf16(x):
    return np.ascontiguousarray(np.asarray(x, dtype=np.float32)).astype(
        ml_dtypes.bfloat16)


def prep_in_maps(key, value, query, mask, Wk, bk, Wq, bq, w_score, b_score,
                 Wf, bf):
    key = np.ascontiguousarray(key, dtype=np.float32)
    query = np.ascontiguousarray(query, dtype=np.float32)

    w16 = np.zeros((8, 128, H), np.float32)
    for c in range(8):
        w16[c, :64, 2 * c] = w_score
        w16[c, 64:, 2 * c + 1] = w_score
    w16 = np.concatenate(list(w16), axis=1)  # [128, 8*16]
    diagmask = np.zeros((H, DK), np.float32)
    for h in range(H):
        diagmask[h, h * 64:(h + 1) * 64] = 1.0
    qkb = (bq + bk).astype(np.float32)
    qkb4 = np.broadcast_to(qkb, (BL, DM)).copy()
    bf4 = np.broadcast_to(bf.astype(np.float32), (BL, DK)).copy()
    maskneg = (mask.astype(np.float32) * np.float32(-1e30))  # [B, LK]

    wk_b, wq_b, wf_b, w16_b = _bf16(Wk), _bf16(Wq), _bf16(Wf), _bf16(w16)
    val_b = _bf16(value)

    in_maps = []
    for i in range(NCORES):
        s = slice(i * BL, (i + 1) * BL)
        in_maps.append({
            "keyT": _bf16(key[s].transpose(0, 2, 1)),
            "value": val_b[s],
            "query": query[s, 0, :],
            "maskneg": np.ascontiguousarray(
                np.broadcast_to(maskneg[s][:, None, :], (BL, H, LK))),
            "Wk": wk_b,
            "Wq": wq_b,
            "Wf": wf_b,
            "w16": w16_b,
            "qkb4": qkb4,
            "bf4": bf4,
            "diagmask": diagmask,
        })
    return in_maps


def kernel(key, value, query, mask, Wk, bk, Wq, bq, w_score, b_score, Wf, bf):
    nc = _get_nc()
    in_maps = prep_in_maps(key, value, query, mask, Wk, bk, Wq, bq,
                           w_score, b_score, Wf, bf)
    res = run_bass_kernel_spmd(nc, in_maps, list(range(NCORES)))
    out = np.concatenate([res.results[i]["out"] for i in range(NCORES)], axis=0)
    attn = np.concatenate([res.results[i]["attn"] for i in range(NCORES)], axis=0)
    return out.reshape(B, 1, DK), attn.reshape(B, H, 1, LK)


# revision 13
# speedup vs baseline: 1.2216x; 1.0302x over previous
"""Multi-head MLP (additive) attention on 8 TRN2 NeuronCores.

Data-parallel over batch: B=32 -> 4 batches per core. Each core computes
its shard fully (no collectives); host scatters inputs / gathers outputs.

Host prep: key is transposed to [B, Dk, Lk] and all matmul operands are
converted to bf16 (halves DMA bytes, full-rate PE streaming).

Per-core pipeline (b = local batch, L=2048 keys, Dm=1024, H=16, d=64):
  key_up^T[c]  = Wk[:,c].T @ key^T                 (bf16 matmuls, PSUM f32)
  tanh_t       = tanh(key_up^T + (q@Wq+bq+bk)^T)   (ScalarE, fused bias)
  scores       = W16c.T @ tanh_t  (accum over c)   (per-head dot, 16 rows)
  attn         = softmax(scores + mask*-1e30)      (DVE/ACT, 16 partitions)
  ctx_full     = attn @ value                      ([16,1024], diag-masked)
  out          = ctx_flat @ Wf + bf                (batched over 4 local b)

b_score is dropped: softmax is shift-invariant.
"""

import ml_dtypes
import numpy as np

import concourse.bass as bass
import concourse.mybir as mybir
import concourse.tile as tile
from concourse import bacc
from concourse.bass_utils import run_bass_kernel_spmd
from concourse.masks import make_identity

F32 = mybir.dt.float32
BF16 = mybir.dt.bfloat16
AF = mybir.ActivationFunctionType
ALU = mybir.AluOpType
AX = mybir.AxisListType

NCORES = 8
B, LK, DK, DM, H = 32, 2048, 1024, 1024, 16
BL = B // NCORES  # 4 local batches per core


def build_program():
    nc = bacc.Bacc("TRN2", target_bir_lowering=False, num_devices=NCORES)

    keyT_d = nc.dram_tensor("keyT", [BL, DK, LK], BF16, kind="ExternalInput").ap()
    val_d = nc.dram_tensor("value", [BL, LK, DK], BF16, kind="ExternalInput").ap()
    q_d = nc.dram_tensor("query", [BL, DM], F32, kind="ExternalInput").ap()
    mneg_d = nc.dram_tensor("maskneg", [BL, H, LK], F32, kind="ExternalInput").ap()
    wk_d = nc.dram_tensor("Wk", [DK, DM], BF16, kind="ExternalInput").ap()
    wq_d = nc.dram_tensor("Wq", [DM, DM], BF16, kind="ExternalInput").ap()
    wf_d = nc.dram_tensor("Wf", [DK, DK], BF16, kind="ExternalInput").ap()
    w16_d = nc.dram_tensor("w16", [128, 8 * H], BF16, kind="ExternalInput").ap()
    qkb_d = nc.dram_tensor("qkb4", [BL, DM], F32, kind="ExternalInput").ap()
    bf4_d = nc.dram_tensor("bf4", [BL, DK], F32, kind="ExternalInput").ap()
    diag_d = nc.dram_tensor("diagmask", [H, DK], F32, kind="ExternalInput").ap()

    out_d = nc.dram_tensor("out", [BL, DK], F32, kind="ExternalOutput").ap()
    attn_d = nc.dram_tensor("attn", [BL, H, LK], F32, kind="ExternalOutput").ap()

    with tile.TileContext(nc) as tc:
        _body(tc, nc, keyT_d, val_d, q_d, mneg_d, wk_d, wq_d, wf_d, w16_d,
              qkb_d, bf4_d, diag_d, out_d, attn_d)

    nc.compile()
    return nc


def _body(tc, nc, keyT_d, val_d, q_d, mneg_d, wk_d, wq_d, wf_d, w16_d,
          qkb_d, bf4_d, diag_d, out_d, attn_d):
    from contextlib import ExitStack

    ctx = ExitStack()
    const = ctx.enter_context(tc.tile_pool(name="const", bufs=1))
    bigw2 = ctx.enter_context(tc.tile_pool(name="bigw2", bufs=1))
    wkp = ctx.enter_context(tc.tile_pool(name="wkp", bufs=1))
    ktp = ctx.enter_context(tc.tile_pool(name="ktp", bufs=3))
    vp = ctx.enter_context(tc.tile_pool(name="vp", bufs=18))
    thp = ctx.enter_context(tc.tile_pool(name="thp", bufs=3))
    scp = ctx.enter_context(tc.tile_pool(name="scp", bufs=2))
    atp = ctx.enter_context(tc.tile_pool(name="atp", bufs=2))
    aTp = ctx.enter_context(tc.tile_pool(name="aTp", bufs=3))
    mkp = ctx.enter_context(tc.tile_pool(name="mkp", bufs=2))
    smp = ctx.enter_context(tc.tile_pool(name="smp", bufs=4))
    cmp_ = ctx.enter_context(tc.tile_pool(name="cmp", bufs=2))
    pku = ctx.enter_context(tc.tile_pool(name="pku", bufs=3, space="PSUM"))
    ptp = ctx.enter_context(tc.tile_pool(name="ptp", bufs=2, space="PSUM"))
    psc = ctx.enter_context(tc.tile_pool(name="psc", bufs=1, space="PSUM"))
    pcx = ctx.enter_context(tc.tile_pool(name="pcx", bufs=1, space="PSUM"))

    # ---- constants ----
    ident = const.tile([128, 128], F32)
    make_identity(nc, ident[:])
    w16_sb = const.tile([128, 8 * H], BF16)
    nc.sync.dma_start(w16_sb[:], w16_d[:])
    diag_sb = const.tile([H, DK], F32)
    nc.sync.dma_start(diag_sb[:], diag_d[:])
    qkb_sb = const.tile([BL, DM], F32)
    nc.sync.dma_start(qkb_sb[:], qkb_d[:])
    bf4_sb = const.tile([BL, DK], F32)
    nc.sync.dma_start(bf4_sb[:], bf4_d[:])
    qTb = const.tile([128, 8 * BL], F32)    # tanh bias, [128, c*4+b]
    ctxT = const.tile([128, 8 * BL], F32)   # ctx_flat^T accum, f32
    ctxTr = const.tile([128, 8 * BL], BF16)  # rounded for final matmul

    # ---- query projection (prologue; must finish before first tanh) ----
    wq_sb = bigw2.tile([128, 8 * DM], BF16, tag="bigw2", name="wq_sb")
    for k in range(8):
        nc.sync.dma_start(wq_sb[:, k * DM:(k + 1) * DM], wq_d[k * 128:(k + 1) * 128, :])
    q_sb = const.tile([BL, DM], F32)
    nc.sync.dma_start(q_sb[:], q_d[:])

    qT = const.tile([128, 8 * BL], BF16)
    for c in range(8):
        pt = ptp.tile([128, 16], F32, tag="pt", name="pt_q")
        nc.tensor.transpose(pt[:, :BL], q_sb[:, c * 128:(c + 1) * 128], ident[:BL, :BL])
        nc.vector.tensor_copy(out=qT[:, c * BL:(c + 1) * BL], in_=pt[:, :BL])

    psq = pcx.tile([BL, DM], F32, tag="c", name="psq")
    for c in range(8):
        for hf in range(2):
            nc.tensor.matmul(
                psq[:, hf * 512:(hf + 1) * 512],
                qT[:, c * BL:(c + 1) * BL],
                wq_sb[:, c * DM + hf * 512: c * DM + (hf + 1) * 512],
                start=(c == 0), stop=(c == 7),
            )
    qsum = const.tile([BL, DM], F32)
    nc.vector.tensor_tensor(qsum[:], psq[:], qkb_sb[:], ALU.add)
    for c in range(8):
        pt = ptp.tile([128, 16], F32, tag="pt", name="pt_qb")
        nc.tensor.transpose(pt[:, :BL], qsum[:, c * 128:(c + 1) * 128], ident[:BL, :BL])
        nc.vector.tensor_copy(out=qTb[:, c * BL:(c + 1) * BL], in_=pt[:, :BL])

    # ---- Wk ----
    wk_sb = wkp.tile([128, 8 * DM], BF16)
    for k in range(8):
        nc.sync.dma_start(wk_sb[:, k * DM:(k + 1) * DM], wk_d[k * 128:(k + 1) * 128, :])

    # ---- main loop over local batches ----
    for b in range(BL):
        mneg = mkp.tile([H, LK], F32)
        nc.sync.dma_start(mneg[:], mneg_d[b])
        scores = scp.tile([H, LK], F32)

        for j in range(4):  # L chunks of 512
            kT = ktp.tile([128, 8 * 512], BF16, tag="kT")
            for k in range(8):
                nc.sync.dma_start(
                    kT[:, k * 512:(k + 1) * 512],
                    keyT_d[b, k * 128:(k + 1) * 128, j * 512:(j + 1) * 512],
                )
            ps = psc.tile([H, 512], F32, tag="s")
            for c in range(8):
                pk = pku.tile([128, 512], F32, tag="t", name="pk")
                for k in range(8):
                    nc.tensor.matmul(
                        pk[:],
                        wk_sb[:, k * DM + c * 128: k * DM + (c + 1) * 128],
                        kT[:, k * 512:(k + 1) * 512],
                        start=(k == 0), stop=(k == 7),
                    )
                th = thp.tile([128, 512], BF16)
                nc.scalar.activation(
                    th[:], pk[:], AF.Tanh,
                    bias=qTb[:, c * BL + b: c * BL + b + 1], scale=1.0,
                )
                nc.tensor.matmul(
                    ps[:], w16_sb[:, c * H:(c + 1) * H], th[:],
                    start=(c == 0), stop=(c == 7),
                )
            nc.vector.tensor_tensor(
                scores[:, j * 512:(j + 1) * 512], ps[:],
                mneg[:, j * 512:(j + 1) * 512], ALU.add,
            )

        vts = []
        for j2 in range(16):
            v = vp.tile([128, DK], BF16, tag="v", name="v")
            nc.sync.dma_start(v[:], val_d[b, j2 * 128:(j2 + 1) * 128, :])
            vts.append(v)

        # softmax over L (free axis), 16 head-partitions
        negmax = smp.tile([H, 1], F32)
        nc.vector.reduce_max(negmax[:], scores[:], axis=AX.X, negate=True)
        attn = atp.tile([H, LK], F32)
        sums = smp.tile([H, 1], F32)
        nc.scalar.activation(attn[:], scores[:], AF.Exp, bias=negmax[:],
                             accum_out=sums[:])
        recip = smp.tile([H, 1], F32)
        nc.vector.reciprocal(recip[:], sums[:])
        nc.vector.tensor_scalar_mul(attn[:], attn[:], recip[:])
        nc.sync.dma_start(attn_d[b], attn[:])

        # context: ctx_full[16,1024] = sum_j2 attnT[j2].T @ value[j2]
        pc = pcx.tile([H, DK], F32, tag="c", name="pc")
        for j2 in range(16):
            pt = ptp.tile([128, 16], F32, tag="pt", name="pt_a")
            nc.tensor.transpose(
                pt[:, :H], attn[:, j2 * 128:(j2 + 1) * 128], ident[:H, :H])
            aT = aTp.tile([128, H], BF16)
            nc.vector.tensor_copy(out=aT[:], in_=pt[:, :H])
            v = vts[j2]
            for hf in range(2):
                nc.tensor.matmul(
                    pc[:, hf * 512:(hf + 1) * 512], aT[:],
                    v[:, hf * 512:(hf + 1) * 512],
                    start=(j2 == 0), stop=(j2 == 15),
                )
        ctxm = cmp_.tile([H, DK], F32)
        nc.vector.tensor_tensor(ctxm[:], pc[:], diag_sb[:], ALU.mult)
        for c in range(8):
            pt = ptp.tile([128, 16], F32, tag="pt", name="pt_c")
            nc.tensor.transpose(
                pt[:, :H], ctxm[:, c * 128:(c + 1) * 128], ident[:H, :H])
            nc.vector.reduce_sum(
                ctxT[:, c * BL + b: c * BL + b + 1], pt[:, :H], axis=AX.X)

    # ---- final projection: out[4,1024] = ctx_flat @ Wf + bf ----
    nc.vector.tensor_copy(out=ctxTr[:], in_=ctxT[:])
    wf_sb = bigw2.tile([128, 8 * DK], BF16, tag="bigw2", name="wf_sb")
    for k in range(8):
        nc.sync.dma_start(wf_sb[:, k * DK:(k + 1) * DK], wf_d[k * 128:(k + 1) * 128, :])
    po = pcx.tile([BL, DK], F32, tag="c", name="po")
    for c in range(8):
        for hf in range(2):
            nc.tensor.matmul(
                po[:, hf * 512:(hf + 1) * 512],
                ctxTr[:, c * BL:(c + 1) * BL],
                wf_sb[:, c * DK + hf * 512: c * DK + (hf + 1) * 512],
                start=(c == 0), stop=(c == 7),
            )
    out_sb = const.tile([BL, DK], F32)
    nc.vector.tensor_tensor(out_sb[:], po[:], bf4_sb[:], ALU.add)
    nc.sync.dma_start(out_d[:], out_sb[:])

    ctx.close()


_CACHED_NC = None


def _get_nc():
    global _CACHED_NC
    if _CACHED_NC is None:
        _CACHED_NC = build_program()
    return _CACHED_NC


def _bf16(x):
    return np.ascontiguousarray(np.asarray(x, dtype=np.float32)).astype(
        ml_dtypes.bfloat16)


def prep_in_maps(key, value, query, mask, Wk, bk, Wq, bq, w_score, b_score,
                 Wf, bf):
    key = np.ascontiguousarray(key, dtype=np.float32)
    query = np.ascontiguousarray(query, dtype=np.float32)

    w16 = np.zeros((8, 128, H), np.float32)
    for c in range(8):
        w16[c, :64, 2 * c] = w_score
        w16[c, 64:, 2 * c + 1] = w_score
    w16 = np.concatenate(list(w16), axis=1)  # [128, 8*16]
    diagmask = np.zeros((H, DK), np.float32)
    for h in range(H):
        diagmask[h, h * 64:(h + 1) * 64] = 1.0
    qkb = (bq + bk).astype(np.float32)
    qkb4 = np.broadcast_to(qkb, (BL, DM)).copy()
    bf4 = np.broadcast_to(bf.astype(np.float32), (BL, DK)).copy()
    maskneg = (mask.astype(np.float32) * np.float32(-1e30))  # [B, LK]

    wk_b, wq_b, wf_b, w16_b = _bf16(Wk), _bf16(Wq), _bf16(Wf), _bf16(w16)
    val_b = _bf16(value)

    in_maps = []
    for i in range(NCORES):
        s = slice(i * BL, (i + 1) * BL)
        in_maps.append({
            "keyT": _bf16(key[s].transpose(0, 2, 1)),
            "value": val_b[s],
            "query": query[s, 0, :],
            "maskneg": np.ascontiguousarray(
                np.broadcast_to(maskneg[s][:, None, :], (BL, H, LK))),
            "Wk": wk_b,
            "Wq": wq_b,
            "Wf": wf_b,
            "w16": w16_b,
            "qkb4": qkb4,
            "bf4": bf4,
            "diagmask": diagmask,
        })
    return in_maps


def kernel(key, value, query, mask, Wk, bk, Wq, bq, w_score, b_score, Wf, bf):
    nc = _get_nc()
    in_maps = prep_in_maps(key, value, query, mask, Wk, bk, Wq, bq,
                           w_score, b_score, Wf, bf)
    res = run_bass_kernel_spmd(nc, in_maps, list(range(NCORES)))
    out = np.concatenate([res.results[i]["out"] for i in range(NCORES)], axis=0)
    attn = np.concatenate([res.results[i]["attn"] for i in range(NCORES)], axis=0)
    return out.reshape(B, 1, DK), attn.reshape(B, H, 1, LK)
